# revision 21
# baseline (speedup 1.0000x reference)
"""Multi-head attention TRN2 kernel (B=4, T=2048, C=1024, H=16, D=64).

Sharding: 8 cores = 4 batches x 2 head-halves. Core c handles batch c//2 and
heads (c%2)*8 .. (c%2)*8+8 (512 of the 1024 channel columns). Each core
computes a partial output projection; the host sums the two partials per
batch and adds the bp / bv rank-1 terms.

Per-core dataflow (one NeuronCore, no collectives), single Tile program
scheduled by the priority-driven list scheduler:
  - Projections (q/k/v) emitted at low priority: they fill PE gaps while
    attention (high priority) is ACT-bound on the exp.
  - Scores per head pair use 2-way PE row tiling: head A occupies array
    rows 0-63 (K=64), head B rows 64-127, running concurrently -> both
    heads' scores cost ~one N=512 matmul.
  - exp on ACT (scale=0.125 folded in, no max subtraction; |scores| < ~8
    so fp32 exp is safe), slabs of TKG=2 key chunks.
  - y^T accumulated on PE with a ones-augmented v ([128,8,65] per key
    chunk) so row 64 is the softmax denominator. reciprocal_approx_fast
    on DVE, partition-broadcast on GPSIMD, normalize+bf16 on DVE.
  - Output projection per 512-wide tq block as soon as all 4 head pairs
    finished that block; fp32 DMA out.
"""

import sys
from contextlib import ExitStack

import numpy as np

sys.path.insert(0, "/opt/trn_rl_repo")

import ml_dtypes  # noqa: E402

import concourse.bass as bass  # noqa: E402
import concourse.bacc as bacc  # noqa: E402
import concourse.mybir as mybir  # noqa: E402
import concourse.tile as tile  # noqa: E402
from concourse.bass_utils import run_bass_kernel_spmd  # noqa: E402

B, T, C, H, D = 4, 2048, 1024, 16, 64
HPC = 8          # heads per core
CC = HPC * D     # per-core channel columns = 512
NCORES = 8
BF16 = mybir.dt.bfloat16
F32 = mybir.dt.float32
BLK = 512        # tq block width
TKG = 2          # tk chunks per exp slab

KC = C // 128    # 8 contraction chunks over C
MC = CC // 128   # 4 column chunks of the per-core 512 cols
NB = T // BLK    # 4 tq blocks
TC = T // 128    # 16 tk chunks

_nc_cache = {}


def _build_nc():
    if "nc" in _nc_cache:
        return _nc_cache["nc"]
    nc = bacc.Bacc("TRN2", target_bir_lowering=False, debug=False)

    xT_d = nc.dram_tensor("xT", [C, T], BF16, kind="ExternalInput").ap()
    wq_d = nc.dram_tensor("wq", [C, CC], BF16, kind="ExternalInput").ap()
    wk_d = nc.dram_tensor("wk", [C, CC], BF16, kind="ExternalInput").ap()
    wv_d = nc.dram_tensor("wv", [C, CC], BF16, kind="ExternalInput").ap()
    wp_d = nc.dram_tensor("wp", [CC, C], BF16, kind="ExternalInput").ap()
    bq_d = nc.dram_tensor("bq2", [128, 4], F32, kind="ExternalInput").ap()
    bk_d = nc.dram_tensor("bk2", [128, 4], F32, kind="ExternalInput").ap()
    out_d = nc.dram_tensor("out", [T, C], F32, kind="ExternalOutput").ap()

    with tile.TileContext(nc) as tc, ExitStack() as ctx:
        p_x = ctx.enter_context(tc.tile_pool(name="x", bufs=KC * NB))
        p_w = ctx.enter_context(tc.tile_pool(name="w", bufs=3 * KC))
        p_wp = ctx.enter_context(tc.tile_pool(name="wp", bufs=MC))
        p_qt = ctx.enter_context(tc.tile_pool(name="qt", bufs=MC * NB))
        p_kp = ctx.enter_context(tc.tile_pool(name="kp", bufs=MC * NB))
        p_v = ctx.enter_context(tc.tile_pool(name="v", bufs=TC))
        p_y = ctx.enter_context(tc.tile_pool(name="yn", bufs=MC * NB))
        p_b = ctx.enter_context(tc.tile_pool(name="bias", bufs=1))
        p_exp = ctx.enter_context(tc.tile_pool(name="exp", bufs=6))
        p_sm = ctx.enter_context(tc.tile_pool(name="sm", bufs=1))
        p_st = ctx.enter_context(tc.tile_pool(name="stage", bufs=1))
        ps_acc = ctx.enter_context(tc.tile_pool(name="pacc", bufs=2, space="PSUM"))
        ps_sc = ctx.enter_context(tc.tile_pool(name="psc", bufs=1, space="PSUM"))
        ps_y = ctx.enter_context(tc.tile_pool(name="psy", bufs=1, space="PSUM"))

        # ---- input DMAs, ordered so the k/q projections of the first
        # column block can begin as early as possible.
        bq2 = p_b.tile([128, MC], F32, tag="bq")
        nc.sync.dma_start(bq2[:], bq_d[:])
        bk2 = p_b.tile([128, MC], F32, tag="bk")
        nc.sync.dma_start(bk2[:], bk_d[:])
        xt = [[None] * NB for _ in range(KC)]
        for k in range(KC):
            t_ = p_x.tile([128, BLK], BF16, tag="xt", name=f"xt{k}_0")
            nc.sync.dma_start(t_[:], xT_d[k * 128:(k + 1) * 128, 0:BLK])
            xt[k][0] = t_
        wk = []
        for k in range(KC):
            t_ = p_w.tile([128, CC], BF16, tag="wk")
            nc.sync.dma_start(t_[:], wk_d[k * 128:(k + 1) * 128, :])
            wk.append(t_)
        wq = []
        for k in range(KC):
            t_ = p_w.tile([128, CC], BF16, tag="wq")
            nc.sync.dma_start(t_[:], wq_d[k * 128:(k + 1) * 128, :])
            wq.append(t_)
        wv = []
        for k in range(KC):
            t_ = p_w.tile([128, CC], BF16, tag="wv")
            nc.sync.dma_start(t_[:], wv_d[k * 128:(k + 1) * 128, :])
            wv.append(t_)
        for b in range(1, NB):
            for k in range(KC):
                t_ = p_x.tile([128, BLK], BF16, tag="xt", name=f"xt{k}_{b}")
                nc.sync.dma_start(
                    t_[:], xT_d[k * 128:(k + 1) * 128, b * BLK:(b + 1) * BLK])
                xt[k][b] = t_
        wp = []
        for k in range(MC):
            t_ = p_wp.tile([128, C], BF16, tag="wp")
            nc.sync.dma_start(t_[:], wp_d[k * 128:(k + 1) * 128, :])
            wp.append(t_)

        # ---- projections (low priority filler; scheduler pulls them
        # as their DMAs land and PE has no higher-priority work).
        kp = [[None] * NB for _ in range(MC)]
        qt = [[None] * NB for _ in range(MC)]
        ytn = [[None] * NB for _ in range(MC)]
        for m in range(MC):
            for b in range(NB):
                kp[m][b] = p_kp.tile([128, BLK], BF16, tag="kp", name=f"kp{m}_{b}")
                qt[m][b] = p_qt.tile([128, BLK], BF16, tag="qt", name=f"qt{m}_{b}")
                ytn[m][b] = p_y.tile([128, BLK], BF16, tag="ytn", name=f"ytn{m}_{b}")

        def proj(dst_ap, wmat, m, b, bias):
            acc = ps_acc.tile([128, BLK], F32, tag="acc")
            for k in range(KC):
                nc.tensor.matmul(
                    acc[:],
                    wmat[k][:, m * 128:(m + 1) * 128],
                    xt[k][b][:],
                    start=(k == 0), stop=(k == KC - 1),
                )
            nc.vector.tensor_scalar_add(dst_ap, acc[:], bias[:, m:m + 1])

        for m in range(MC):
            for b in range(NB):
                proj(kp[m][b][:], wk, m, b, bk2)
            if m == 0:
                for b in range(NB):
                    proj(qt[0][b][:], wq, 0, b, bq2)
                # v: natural layout, interleaved per head, ones col appended
                vaug = []
                for t_ in range(TC):
                    va = p_v.tile([128, HPC, D + 1], BF16, tag="va")
                    vaug.append(va)
                    nc.gpsimd.memset(va[:, :, D:D + 1], 1.0)
                    acc = ps_acc.tile([128, CC], F32, tag="acc")
                    for k in range(KC):
                        nc.tensor.matmul(
                            acc[:],
                            xt[k][t_ // 4][:, (t_ % 4) * 128:(t_ % 4 + 1) * 128],
                            wv[k][:],
                            start=(k == 0), stop=(k == KC - 1),
                        )
                    nc.vector.tensor_copy(
                        va[:, :, 0:D], acc[:].rearrange("p (h d) -> p h d", d=D))
            else:
                for b in range(NB):
                    proj(qt[m][b][:], wq, m, b, bq2)

        # ---- attention, high priority: one global window stream of
        # (pair, blk, key-chunk). The scores+exp chain leads (s double-
        # buffered -> S(j+1) streams during E(j), exp runs gapless); AV
        # matmuls trail by LAG windows as lower-priority PE work; each
        # iteration's tail AVs + normalize are deferred into the next
        # iteration so they never block the next scores/exp chain.
        LAG = 2

        def make_tail(c, blk, y0, y1, ees):
            def tail():
                for tk in range(TC - LAG, TC):
                    nc.tensor.matmul(
                        y0[:], vaug[tk][:, 2 * c, :], ees[tk][:, 0, :],
                        start=(tk == 0), stop=(tk == TC - 1))
                    nc.tensor.matmul(
                        y1[:], vaug[tk][:, 2 * c + 1, :], ees[tk][:, 1, :],
                        start=(tk == 0), stop=(tk == TC - 1))
                for half, yy in ((0, y0), (1, y1)):
                    ys = p_sm.tile([D, BLK], F32, tag=f"ys{half}")
                    nc.vector.tensor_copy(ys[:], yy[0:D, :])
                    # denominator row to a base-0 tile: the custom-DVE
                    # reciprocal mishandles inputs at partition base 64
                    dn = p_sm.tile([1, BLK], F32, tag=f"d{half}")
                    nc.vector.tensor_copy(dn[:], yy[D:D + 1, :])
                    rr = p_sm.tile([1, BLK], F32, tag=f"r{half}")
                    nc.vector.reciprocal_approx_fast(out=rr[:], in_=dn[:])
                    bb = p_sm.tile([D, BLK], F32, tag=f"b{half}")
                    nc.gpsimd.partition_broadcast(bb[:], rr[:])
                    nc.vector.tensor_mul(
                        ytn[c][blk][half * 64:half * 64 + 64, :],
                        ys[:], bb[:])
            return tail

        deferred = None
        with tc.high_priority():
            for blk in range(NB):
                for c in range(MC):          # head pair (2c, 2c+1)
                    y0 = ps_y.tile([D + 1, BLK], F32, tag="y0")
                    y1 = ps_y.tile([D + 1, BLK], F32, tag="y1")
                    ees = []
                    for tk in range(TC):
                        if tk == LAG and deferred is not None:
                            deferred()
                            deferred = None
                        if tk >= LAG:
                            j = tk - LAG
                            nc.tensor.matmul(
                                y0[:], vaug[j][:, 2 * c, :], ees[j][:, 0, :],
                                start=(j == 0), stop=(j == TC - 1))
                            nc.tensor.matmul(
                                y1[:], vaug[j][:, 2 * c + 1, :],
                                ees[j][:, 1, :],
                                start=(j == 0), stop=(j == TC - 1))
                        kb, ko = divmod(tk, 4)
                        tcols = slice(ko * 128, (ko + 1) * 128)
                        # one combined [128, 2, BLK] slab per key chunk:
                        # head A via array rows 0-63, head B rows 64-127
                        # (2-way row tiling; both MMs become ready on the
                        # same exp completion, so they stream concurrently)
                        ss = ps_sc.tile([128, 2, BLK], F32, tag="s")
                        nc.tensor.matmul(
                            ss[:, 0, :], kp[c][kb][0:64, tcols],
                            qt[c][blk][0:64, :], start=True, stop=True)
                        nc.tensor.matmul(
                            ss[:, 1, :], kp[c][kb][64:128, tcols],
                            qt[c][blk][64:128, :], start=True, stop=True)
                        ee = p_exp.tile([128, 2, BLK], BF16, tag="e")
                        nc.scalar.activation(
                            ee[:], ss[:],
                            mybir.ActivationFunctionType.Exp, scale=0.125)
                        ees.append(ee)
                    deferred = make_tail(c, blk, y0, y1, ees)
            deferred()

        # ---- output projection: emitted last at low priority, becomes
        # PE filler once each block's ytn tiles are normalized.
        for blk in range(NB):
            for tq in range(BLK // 128):
                rows = slice(blk * BLK + tq * 128, blk * BLK + (tq + 1) * 128)
                rloc = slice(tq * 128, (tq + 1) * 128)
                for half in range(2):
                    po = ps_acc.tile([128, BLK], F32, tag="acc")
                    for k in range(MC):
                        nc.tensor.matmul(
                            po[:],
                            ytn[k][blk][:, rloc],
                            wp[k][:, half * BLK:(half + 1) * BLK],
                            start=(k == 0), stop=(k == MC - 1))
                    st = p_st.tile([128, BLK], F32, tag="st")
                    nc.vector.tensor_copy(st[:], po[:])
                    nc.sync.dma_start(
                        out_d[rows, half * BLK:(half + 1) * BLK], st[:])

    nc.compile()
    _nc_cache["nc"] = nc
    return nc


def prepare_in_maps(x, Wq, bq, Wk, bk, Wv, bv, Wp, bp):
    x = np.asarray(x, dtype=np.float32)
    Wq, bq = np.asarray(Wq, np.float32), np.asarray(bq, np.float32)
    Wk, bk = np.asarray(Wk, np.float32), np.asarray(bk, np.float32)
    Wv = np.asarray(Wv, np.float32)
    Wp = np.asarray(Wp, np.float32)
    bf = ml_dtypes.bfloat16

    in_maps = []
    for c in range(NCORES):
        b, half = divmod(c, 2)
        cols = slice(half * CC, (half + 1) * CC)
        in_maps.append({
            "xT": np.ascontiguousarray(x[b].T).astype(bf),
            "wq": np.ascontiguousarray(Wq[:, cols]).astype(bf),
            "wk": np.ascontiguousarray(Wk[:, cols]).astype(bf),
            "wv": np.ascontiguousarray(Wv[:, cols]).astype(bf),
            "wp": np.ascontiguousarray(Wp[cols, :]).astype(bf),
            "bq2": np.ascontiguousarray(bq[cols].reshape(4, 128).T),
            "bk2": np.ascontiguousarray(bk[cols].reshape(4, 128).T),
        })
    return in_maps


def combine(results, Wv, bv, Wp, bp):
    bv = np.asarray(bv, np.float32)
    Wp = np.asarray(Wp, np.float32)
    bp = np.asarray(bp, np.float32)
    out = np.zeros((B, T, C), np.float32)
    for c in range(NCORES):
        b, half = divmod(c, 2)
        cols = slice(half * CC, (half + 1) * CC)
        out[b] += results[c]["out"]
        # bv enters y as att@1 * bv = bv per row (softmax rows sum to 1)
        out[b] += bv[cols] @ Wp[cols, :]
    out += bp
    return out


def kernel(x, Wq, bq, Wk, bk, Wv, bv, Wp, bp):
    in_maps = prepare_in_maps(x, Wq, bq, Wk, bk, Wv, bv, Wp, bp)
    nc = _build_nc()
    res = run_bass_kernel_spmd(nc, in_maps, list(range(NCORES))).results
    return combine(res, Wv, bv, Wp, bp)


# revision 22
# speedup vs baseline: 1.2537x; 1.2537x over previous
"""Multi-head attention TRN2 kernel (B=4, T=2048, C=1024, H=16, D=64).

Sharding: 8 cores = 4 batches x 2 head-halves. Core c handles batch c//2 and
heads (c%2)*8 .. (c%2)*8+8 (512 of the 1024 channel columns). Each core
computes a partial output projection; the host sums the two partials per
batch and adds the bp / bv rank-1 terms.

Per-core dataflow (one NeuronCore, no collectives), single Tile program
scheduled by the priority-driven list scheduler:
  - Projections (q/k/v) emitted at low priority: they fill PE gaps while
    attention (high priority) is ACT-bound on the exp.
  - Scores per head pair use 2-way PE row tiling: head A occupies array
    rows 0-63 (K=64), head B rows 64-127, running concurrently -> both
    heads' scores cost ~one N=512 matmul.
  - exp on ACT (scale=0.125 folded in, no max subtraction; |scores| < ~8
    so fp32 exp is safe), slabs of TKG=2 key chunks.
  - y^T accumulated on PE with a ones-augmented v ([128,8,65] per key
    chunk) so row 64 is the softmax denominator. reciprocal_approx_fast
    on DVE, partition-broadcast on GPSIMD, normalize+bf16 on DVE.
  - Output projection per 512-wide tq block as soon as all 4 head pairs
    finished that block; fp32 DMA out.
"""

import sys
from contextlib import ExitStack

import numpy as np

sys.path.insert(0, "/opt/trn_rl_repo")

import ml_dtypes  # noqa: E402

import concourse.bass as bass  # noqa: E402
import concourse.bacc as bacc  # noqa: E402
import concourse.mybir as mybir  # noqa: E402
import concourse.tile as tile  # noqa: E402
from concourse.bass_utils import run_bass_kernel_spmd  # noqa: E402

B, T, C, H, D = 4, 2048, 1024, 16, 64
HPC = 8          # heads per core
CC = HPC * D     # per-core channel columns = 512
NCORES = 8
BF16 = mybir.dt.bfloat16
F32 = mybir.dt.float32
BLK = 512        # tq block width
TKG = 2          # tk chunks per exp slab

KC = C // 128    # 8 contraction chunks over C
MC = CC // 128   # 4 column chunks of the per-core 512 cols
NB = T // BLK    # 4 tq blocks
TC = T // 128    # 16 tk chunks

_nc_cache = {}


def _build_nc():
    if "nc" in _nc_cache:
        return _nc_cache["nc"]
    nc = bacc.Bacc("TRN2", target_bir_lowering=False, debug=False)

    xT_d = nc.dram_tensor("xT", [C, T], BF16, kind="ExternalInput").ap()
    wq_d = nc.dram_tensor("wq", [C, CC], BF16, kind="ExternalInput").ap()
    wk_d = nc.dram_tensor("wk", [C, CC], BF16, kind="ExternalInput").ap()
    wv_d = nc.dram_tensor("wv", [C, CC], BF16, kind="ExternalInput").ap()
    wp_d = nc.dram_tensor("wp", [CC, C], BF16, kind="ExternalInput").ap()
    bq_d = nc.dram_tensor("bq2", [128, 4], F32, kind="ExternalInput").ap()
    bk_d = nc.dram_tensor("bk2", [128, 4], F32, kind="ExternalInput").ap()
    out_d = nc.dram_tensor("out", [T, C], F32, kind="ExternalOutput").ap()

    with tile.TileContext(nc) as tc, ExitStack() as ctx:
        p_x = ctx.enter_context(tc.tile_pool(name="x", bufs=KC * NB))
        p_w = ctx.enter_context(tc.tile_pool(name="w", bufs=3 * KC))
        p_wp = ctx.enter_context(tc.tile_pool(name="wp", bufs=MC))
        p_qt = ctx.enter_context(tc.tile_pool(name="qt", bufs=MC * NB))
        p_kp = ctx.enter_context(tc.tile_pool(name="kp", bufs=MC * NB))
        p_v = ctx.enter_context(tc.tile_pool(name="v", bufs=TC))
        p_y = ctx.enter_context(tc.tile_pool(name="yn", bufs=MC * NB))
        p_b = ctx.enter_context(tc.tile_pool(name="bias", bufs=1))
        p_exp = ctx.enter_context(tc.tile_pool(name="exp", bufs=6))
        p_sm = ctx.enter_context(tc.tile_pool(name="sm", bufs=1))
        p_st = ctx.enter_context(tc.tile_pool(name="stage", bufs=1))
        ps_acc = ctx.enter_context(tc.tile_pool(name="pacc", bufs=2, space="PSUM"))
        ps_sc = ctx.enter_context(tc.tile_pool(name="psc", bufs=2, space="PSUM"))
        ps_y = ctx.enter_context(tc.tile_pool(name="psy", bufs=1, space="PSUM"))

        # ---- input DMAs, ordered so the k/q projections of the first
        # column block can begin as early as possible.
        bq2 = p_b.tile([128, MC], F32, tag="bq")
        nc.sync.dma_start(bq2[:], bq_d[:])
        bk2 = p_b.tile([128, MC], F32, tag="bk")
        nc.sync.dma_start(bk2[:], bk_d[:])
        xt = [[None] * NB for _ in range(KC)]
        for k in range(KC):
            t_ = p_x.tile([128, BLK], BF16, tag="xt", name=f"xt{k}_0")
            nc.sync.dma_start(t_[:], xT_d[k * 128:(k + 1) * 128, 0:BLK])
            xt[k][0] = t_
        wk = []
        for k in range(KC):
            t_ = p_w.tile([128, CC], BF16, tag="wk")
            nc.sync.dma_start(t_[:], wk_d[k * 128:(k + 1) * 128, :])
            wk.append(t_)
        wq = []
        for k in range(KC):
            t_ = p_w.tile([128, CC], BF16, tag="wq")
            nc.sync.dma_start(t_[:], wq_d[k * 128:(k + 1) * 128, :])
            wq.append(t_)
        wv = []
        for k in range(KC):
            t_ = p_w.tile([128, CC], BF16, tag="wv")
            nc.sync.dma_start(t_[:], wv_d[k * 128:(k + 1) * 128, :])
            wv.append(t_)
        for b in range(1, NB):
            for k in range(KC):
                t_ = p_x.tile([128, BLK], BF16, tag="xt", name=f"xt{k}_{b}")
                nc.sync.dma_start(
                    t_[:], xT_d[k * 128:(k + 1) * 128, b * BLK:(b + 1) * BLK])
                xt[k][b] = t_
        wp = []
        for k in range(MC):
            t_ = p_wp.tile([128, C], BF16, tag="wp")
            nc.sync.dma_start(t_[:], wp_d[k * 128:(k + 1) * 128, :])
            wp.append(t_)

        # ---- projections (low priority filler; scheduler pulls them
        # as their DMAs land and PE has no higher-priority work).
        kp = [[None] * NB for _ in range(MC)]
        qt = [[None] * NB for _ in range(MC)]
        ytn = [[None] * NB for _ in range(MC)]
        for m in range(MC):
            for b in range(NB):
                kp[m][b] = p_kp.tile([128, BLK], BF16, tag="kp", name=f"kp{m}_{b}")
                qt[m][b] = p_qt.tile([128, BLK], BF16, tag="qt", name=f"qt{m}_{b}")
                ytn[m][b] = p_y.tile([128, BLK], BF16, tag="ytn", name=f"ytn{m}_{b}")

        def proj(dst_ap, wmat, m, b, bias):
            acc = ps_acc.tile([128, BLK], F32, tag="acc")
            for k in range(KC):
                nc.tensor.matmul(
                    acc[:],
                    wmat[k][:, m * 128:(m + 1) * 128],
                    xt[k][b][:],
                    start=(k == 0), stop=(k == KC - 1),
                )
            nc.vector.tensor_scalar_add(dst_ap, acc[:], bias[:, m:m + 1])

        for m in range(MC):
            for b in range(NB):
                proj(kp[m][b][:], wk, m, b, bk2)
            if m == 0:
                for b in range(NB):
                    proj(qt[0][b][:], wq, 0, b, bq2)
                # v: natural layout, interleaved per head, ones col appended
                vaug = []
                for t_ in range(TC):
                    va = p_v.tile([128, HPC, D + 1], BF16, tag="va")
                    vaug.append(va)
                    nc.gpsimd.memset(va[:, :, D:D + 1], 1.0)
                    acc = ps_acc.tile([128, CC], F32, tag="acc")
                    for k in range(KC):
                        nc.tensor.matmul(
                            acc[:],
                            xt[k][t_ // 4][:, (t_ % 4) * 128:(t_ % 4 + 1) * 128],
                            wv[k][:],
                            start=(k == 0), stop=(k == KC - 1),
                        )
                    nc.vector.tensor_copy(
                        va[:, :, 0:D], acc[:].rearrange("p (h d) -> p h d", d=D))
            else:
                for b in range(NB):
                    proj(qt[m][b][:], wq, m, b, bq2)

        # ---- attention, high priority: one global window stream of
        # (pair, blk, key-chunk). The scores+exp chain leads (s double-
        # buffered -> S(j+1) streams during E(j), exp runs gapless); AV
        # matmuls trail by LAG windows as lower-priority PE work; each
        # iteration's tail AVs + normalize are deferred into the next
        # iteration so they never block the next scores/exp chain.
        LAG = 2

        def make_tail(c, blk, y0, y1, ees):
            def tail():
                for tk in range(TC - LAG, TC):
                    nc.tensor.matmul(
                        y0[:], vaug[tk][:, 2 * c, :], ees[tk][:, 0, :],
                        start=(tk == 0), stop=(tk == TC - 1))
                    nc.tensor.matmul(
                        y1[:], vaug[tk][:, 2 * c + 1, :], ees[tk][:, 1, :],
                        start=(tk == 0), stop=(tk == TC - 1))
                for half, yy in ((0, y0), (1, y1)):
                    ys = p_sm.tile([D, BLK], F32, tag=f"ys{half}")
                    nc.vector.tensor_copy(ys[:], yy[0:D, :])
                    # denominator row to a base-0 tile: the custom-DVE
                    # reciprocal mishandles inputs at partition base 64
                    dn = p_sm.tile([1, BLK], F32, tag=f"d{half}")
                    nc.vector.tensor_copy(dn[:], yy[D:D + 1, :])
                    rr = p_sm.tile([1, BLK], F32, tag=f"r{half}")
                    nc.vector.reciprocal_approx_fast(out=rr[:], in_=dn[:])
                    bb = p_sm.tile([D, BLK], F32, tag=f"b{half}")
                    nc.gpsimd.partition_broadcast(bb[:], rr[:])
                    nc.vector.tensor_mul(
                        ytn[c][blk][half * 64:half * 64 + 64, :],
                        ys[:], bb[:])
            return tail

        deferred = None
        with tc.high_priority():
            for blk in range(NB):
                for c in range(MC):          # head pair (2c, 2c+1)
                    y0 = ps_y.tile([D + 1, BLK], F32, tag="y0")
                    y1 = ps_y.tile([D + 1, BLK], F32, tag="y1")
                    ees = []
                    for tk in range(TC):
                        if tk == LAG and deferred is not None:
                            deferred()
                            deferred = None
                        if tk >= LAG:
                            j = tk - LAG
                            nc.tensor.matmul(
                                y0[:], vaug[j][:, 2 * c, :], ees[j][:, 0, :],
                                start=(j == 0), stop=(j == TC - 1))
                            nc.tensor.matmul(
                                y1[:], vaug[j][:, 2 * c + 1, :],
                                ees[j][:, 1, :],
                                start=(j == 0), stop=(j == TC - 1))
                        kb, ko = divmod(tk, 4)
                        tcols = slice(ko * 128, (ko + 1) * 128)
                        # one combined [128, 2, BLK] slab per key chunk:
                        # head A via array rows 0-63, head B rows 64-127
                        # (2-way row tiling; both MMs become ready on the
                        # same exp completion, so they stream concurrently)
                        ss = ps_sc.tile([128, 2, BLK], F32, tag="s")
                        nc.tensor.matmul(
                            ss[:, 0, :], kp[c][kb][0:64, tcols],
                            qt[c][blk][0:64, :], start=True, stop=True)
                        nc.tensor.matmul(
                            ss[:, 1, :], kp[c][kb][64:128, tcols],
                            qt[c][blk][64:128, :], start=True, stop=True)
                        ee = p_exp.tile([128, 2, BLK], BF16, tag="e")
                        nc.scalar.activation(
                            ee[:], ss[:],
                            mybir.ActivationFunctionType.Exp, scale=0.125)
                        ees.append(ee)
                    deferred = make_tail(c, blk, y0, y1, ees)
            deferred()

        # ---- output projection: emitted last at low priority, becomes
        # PE filler once each block's ytn tiles are normalized.
        for blk in range(NB):
            for tq in range(BLK // 128):
                rows = slice(blk * BLK + tq * 128, blk * BLK + (tq + 1) * 128)
                rloc = slice(tq * 128, (tq + 1) * 128)
                for half in range(2):
                    po = ps_acc.tile([128, BLK], F32, tag="acc")
                    for k in range(MC):
                        nc.tensor.matmul(
                            po[:],
                            ytn[k][blk][:, rloc],
                            wp[k][:, half * BLK:(half + 1) * BLK],
                            start=(k == 0), stop=(k == MC - 1))
                    st = p_st.tile([128, BLK], F32, tag="st")
                    nc.vector.tensor_copy(st[:], po[:])
                    nc.sync.dma_start(
                        out_d[rows, half * BLK:(half + 1) * BLK], st[:])

    nc.compile()
    _nc_cache["nc"] = nc
    return nc


def prepare_in_maps(x, Wq, bq, Wk, bk, Wv, bv, Wp, bp):
    x = np.asarray(x, dtype=np.float32)
    Wq, bq = np.asarray(Wq, np.float32), np.asarray(bq, np.float32)
    Wk, bk = np.asarray(Wk, np.float32), np.asarray(bk, np.float32)
    Wv = np.asarray(Wv, np.float32)
    Wp = np.asarray(Wp, np.float32)
    bf = ml_dtypes.bfloat16

    in_maps = []
    for c in range(NCORES):
        b, half = divmod(c, 2)
        cols = slice(half * CC, (half + 1) * CC)
        in_maps.append({
            "xT": np.ascontiguousarray(x[b].T).astype(bf),
            "wq": np.ascontiguousarray(Wq[:, cols]).astype(bf),
            "wk": np.ascontiguousarray(Wk[:, cols]).astype(bf),
            "wv": np.ascontiguousarray(Wv[:, cols]).astype(bf),
            "wp": np.ascontiguousarray(Wp[cols, :]).astype(bf),
            "bq2": np.ascontiguousarray(bq[cols].reshape(4, 128).T),
            "bk2": np.ascontiguousarray(bk[cols].reshape(4, 128).T),
        })
    return in_maps


def combine(results, Wv, bv, Wp, bp):
    bv = np.asarray(bv, np.float32)
    Wp = np.asarray(Wp, np.float32)
    bp = np.asarray(bp, np.float32)
    out = np.zeros((B, T, C), np.float32)
    for c in range(NCORES):
        b, half = divmod(c, 2)
        cols = slice(half * CC, (half + 1) * CC)
        out[b] += results[c]["out"]
        # bv enters y as att@1 * bv = bv per row (softmax rows sum to 1)
        out[b] += bv[cols] @ Wp[cols, :]
    out += bp
    return out


def kernel(x, Wq, bq, Wk, bk, Wv, bv, Wp, bp):
    in_maps = prepare_in_maps(x, Wq, bq, Wk, bk, Wv, bv, Wp, bp)
    nc = _build_nc()
    res = run_bass_kernel_spmd(nc, in_maps, list(range(NCORES))).results
    return combine(res, Wv, bv, Wp, bp)
